# revision 27
# baseline (speedup 1.0000x reference)
"""SAGAN-style attention block (nn_AttentionBlock) on 8 Trainium2 NeuronCores.

Math (per batch b):
    q = wq @ x + bq            [C8, N]
    k = wk @ x + bk            [C8, N]
    v = wv @ x + bv            [C,  N]
    S[n, m]  = sum_o q[o,n] k[o,m]
    attn     = softmax_m(S)
    out[c,n] = sum_m v[c,m] attn[n,m]
    y        = gamma * out + x

Sharding: 8 cores = 4 batches x 2 halves of the n (query-row) axis.

Design notes (final, all from trace evidence):
  - q/k path in fp16 (bf16 there costs 1.5e-2 rel err via exp amplification;
    fp16 gives 2e-3).  P=exp(S) in bf16 (it overflows fp16).
  - wq/wk host-tiled 4x across PE row-groups so the projection replicates
    q/k into all four 32-row partition groups; the quadrant-packed QK^T
    matmuls (tile_position=(32g,0), concurrent on the PE) slice them
    directly - no packing DMAs.
  - V projection eliminated: out = wv @ (x @ P^T) reassociation.  The AV
    matmuls take host-transposed bf16 x^T tiles as weights; a 4-matmul
    out-projection per block applies gamma*wv afterward, reusing the
    accumulator banks after a Z drain.
  - gamma folded into wv on the host; gamma*bv folded into the xq residual.
  - x columns are permuted per core (own query half first) so q-projection
    reads xh chunks 0..3 - no duplicate xqh load.  Attention is
    permutation-invariant over key positions since k and x^T share the
    permutation.
  - Per-chunk SBUF tiles everywhere: Tile tracks deps per tile, so chunked
    DMAs unblock compute as each chunk lands.
  - Denominator: per-slot [128,2048] bf16 running sum on DVE, folded 4->1,
    partition-summed+broadcast by one gpsimd.partition_all_reduce per block
    (hidden under the next block); the last block pre-folds slots 0..6 into
    a PE partition-reduce during slot 7 and corrects with slot 7's fold, so
    only ~3us of tail remains after the final exp.
  - Fast custom-DVE reciprocal (reciprocal_approx_fast, ~1 cyc/elem).
  - PSUM: 4 banks S^T (single buffer) + 2x2 accumulator banks alternating
    between blocks so block nb+1's matmuls overlap block nb's tail.
  - fp32 residual xq late-loads per block, gated behind the previous
    block's all_reduce on the gpsimd queue.
  - ~3.5us of dummy matmuls at t=0 keep the PE HAM window warm while the
    first DMAs land (otherwise projections run at the cold 1.2GHz clock).
"""

import sys

sys.path.insert(0, "/opt/trn_rl_repo")

import numpy as np  # noqa: E402

B, C, HH, WW = 4, 256, 64, 64
N = HH * WW  # 4096
C8 = C // 8  # 32
P = 128
CT = C // P  # 2 channel tiles
NQ = N // 2  # 2048 query rows per core
NBLK = 512  # n-block (query columns per block)
NBLKS = NQ // NBLK  # 4
MT = N // P  # 32 m-tiles (key/value positions)
GRP = 4  # m-tiles per S^T psum slot
NSLOT = MT // GRP  # 8 slots per block
CHUNK = 512
NCHUNKS = N // CHUNK  # 8
QCHUNKS = NQ // CHUNK  # 4
NCORES = 8

_prog = None


def _build(debug_taps=False):
    import concourse.bacc as bacc
    import concourse.bass_isa as bass_isa
    import concourse.mybir as mybir
    import concourse.tile as tile

    f32 = mybir.dt.float32
    f16 = mybir.dt.float16
    bf16 = mybir.dt.bfloat16
    AluAdd = mybir.AluOpType.add
    Exp = mybir.ActivationFunctionType.Exp
    RAdd = bass_isa.ReduceOp.add

    nc = bacc.Bacc("TRN2", target_bir_lowering=False, debug=False)

    dbg = {}
    if debug_taps:
        dbg["dacc"] = nc.dram_tensor("dbg_dacc", [P, GRP * NBLK], bf16, kind="ExternalOutput")
        dbg["dbc"] = nc.dram_tensor("dbg_dbc", [P, NBLK], f32, kind="ExternalOutput")

    # xh/xt are PERMUTED per core on the host: the core's own query half
    # comes first, so q-projection reads xh chunks 0..3.  Attention is
    # permutation-invariant over key positions as long as k and x^T use the
    # same order.
    xh_d = nc.dram_tensor("xh", [C, N], f16, kind="ExternalInput")
    xt_d = nc.dram_tensor("xt", [N, C], bf16, kind="ExternalInput")
    xq_d = nc.dram_tensor("xq", [C, NQ], f32, kind="ExternalInput")
    wqt4_d = nc.dram_tensor("wqt4", [C, P], f16, kind="ExternalInput")
    wkt4_d = nc.dram_tensor("wkt4", [C, P], f16, kind="ExternalInput")
    wvt_d = nc.dram_tensor("wvt", [C, C], bf16, kind="ExternalInput")
    bq4_d = nc.dram_tensor("bq4", [P], f32, kind="ExternalInput")
    bk4_d = nc.dram_tensor("bk4", [P], f32, kind="ExternalInput")
    out_d = nc.dram_tensor("out", [C, NQ], f32, kind="ExternalOutput")

    with tile.TileContext(nc) as tc:
        with (
            tc.tile_pool(name="const", bufs=1) as const,
            tc.tile_pool(name="big", bufs=1) as big,
        ):
            # per-chunk tiles => fine-grained DMA->compute dependencies
            xh_c = [big.tile([P, CT, CHUNK], f16, name=f"xh{i}") for i in range(NCHUNKS)]
            xq_c = [big.tile([P, CT, NBLK], f32, name=f"xq{i}") for i in range(NBLKS)]
            k_c = [big.tile([P, CHUNK], f16, name=f"k{i}") for i in range(NCHUNKS)]
            q_c = [big.tile([P, NBLK], f16, name=f"q{i}") for i in range(NBLKS)]
            xt_c = [big.tile([P, GRP, C], bf16, name=f"xt{i}") for i in range(NSLOT)]

            wqt4 = const.tile([P, CT, P], f16)
            wkt4 = const.tile([P, CT, P], f16)
            wvt = const.tile([P, CT, C], bf16)
            bq4 = const.tile([P, 1], f32)
            bk4 = const.tile([P, 1], f32)
            ones_bf = const.tile([P, 1], bf16)
            nc.vector.memset(ones_bf, 1.0)

            xh_r = xh_d.ap().rearrange("(t p) n -> p t n", p=P)
            xt_r = xt_d.ap().rearrange("(m p) c -> p m c", p=P)
            xq_r = xq_d.ap().rearrange("(t p) n -> p t n", p=P)
            out_r = out_d.ap().rearrange("(t p) n -> p t n", p=P)

            # xh split across both DMA queues (halves projection-phase DMA
            # pacing); xt follows on sync and lands during the exp stream
            nc.sync.dma_start(out=wkt4, in_=wkt4_d.ap().rearrange("(t p) o -> p t o", p=P))
            nc.sync.dma_start(out=bk4, in_=bk4_d.ap()[:, None])
            nc.scalar.dma_start(out=xh_c[1], in_=xh_r[:, :, CHUNK:2 * CHUNK])
            nc.scalar.dma_start(out=wqt4, in_=wqt4_d.ap().rearrange("(t p) o -> p t o", p=P))
            nc.scalar.dma_start(out=bq4, in_=bq4_d.ap()[:, None])
            for i in range(NCHUNKS):
                if i == 1:
                    continue
                sl = slice(i * CHUNK, (i + 1) * CHUNK)
                eng = nc.sync if i % 2 == 0 else nc.scalar
                eng.dma_start(out=xh_c[i], in_=xh_r[:, :, sl])
            for i in range(NCHUNKS):
                nc.sync.dma_start(out=xt_c[i], in_=xt_r[:, GRP * i:GRP * (i + 1), :])
            nc.sync.dma_start(out=wvt, in_=wvt_d.ap().rearrange("(t p) o -> p t o", p=P))

            # ---- fused projection + attention ----
            # PSUM budget: during block 0 the projection pool (2 banks)
            # coexists with S^T (4) and block-0 accumulators (2) = 8; the
            # blocks-1..3 accumulator pool takes the projection banks after.
            with (
                tc.tile_pool(name="st_ps", bufs=1, space="PSUM") as stp,
                tc.tile_pool(name="ptp", bufs=3) as ptp,
                tc.tile_pool(name="dap", bufs=2) as dap,
                tc.tile_pool(name="dnp", bufs=2) as dnp,
                tc.tile_pool(name="finp", bufs=4) as finp,
            ):
                bstate = {}

                def emit_av(nb, mg, pt, skip_dacc=False):
                    accs, dacc, nb_ = bstate[nb]
                    for i in range(GRP):
                        mt = GRP * mg + i
                        for cc in range(CT):
                            nc.tensor.matmul(
                                accs[cc],
                                lhsT=xt_c[mg][:, i, cc * P:(cc + 1) * P],
                                rhs=pt[:, i, :],
                                start=(mt == 0),
                                stop=(mt == MT - 1),
                            )
                    if skip_dacc:
                        return
                    # denominator partial: one 2048-elem bf16 add per slot
                    if mg == 0:
                        nc.vector.tensor_copy(out=dacc, in_=pt)
                    else:
                        nc.vector.tensor_tensor(dacc, dacc, pt, AluAdd)

                def emit_tail(nb):
                    accs, dacc, nb_ = bstate.pop(nb)
                    nsl = slice(nb * NBLK, (nb + 1) * NBLK)
                    # drain Z = x @ P^T to SBUF, then out = wv_g @ Z reusing
                    # the same accumulator banks (WAR dep via Tile)
                    zsb = finp.tile([P, CT, NBLK], bf16, tag="zsb", name="zsb")
                    for cc in range(CT):
                        nc.vector.tensor_copy(out=zsb[:, cc, :], in_=accs[cc])
                    for co in range(CT):
                        for ci in range(CT):
                            nc.tensor.matmul(
                                accs[co],
                                lhsT=wvt[:, ci, co * P:(co + 1) * P],
                                rhs=zsb[:, ci, :],
                                start=(ci == 0),
                                stop=(ci == CT - 1),
                            )
                    d2 = dnp.tile([P, 2, NBLK], bf16, tag="d2", name="d2")
                    nc.vector.tensor_tensor(d2, dacc[:, 0:2, :], dacc[:, 2:4, :], AluAdd)
                    d1 = dnp.tile([P, NBLK], bf16, tag="d1", name="d1")
                    nc.vector.tensor_tensor(d1, d2[:, 0, :], d2[:, 1, :], AluAdd)
                    rec = dnp.tile([P, NBLK], f32, tag="rec", name="rec")
                    # gpsimd sum-over-partitions with broadcast result;
                    # its ~3.5us latency hides under the next block
                    dbc = dnp.tile([P, NBLK], f32, tag="dbc", name="dbc")
                    nc.gpsimd.partition_all_reduce(dbc, d1, channels=P, reduce_op=RAdd)
                    nc.vector.reciprocal_approx_fast(rec, dbc)
                    # gated late-load of the NEXT block's fp32 residual slice
                    # (gpsimd FIFO: issues only once this all_reduce is done)
                    if nb + 1 < NBLKS:
                        nn_ = slice((nb + 1) * NBLK, (nb + 2) * NBLK)
                        nc.gpsimd.dma_start(out=xq_c[nb + 1], in_=xq_r[:, :, nn_])
                    if debug_taps and nb == 0:
                        nc.sync.dma_start(out=dbg["dacc"].ap().rearrange("p (g n) -> p g n", g=GRP), in_=dacc)
                        nc.sync.dma_start(out=dbg["dbc"].ap(), in_=dbc)
                    for cc in range(CT):
                        fin = finp.tile([P, NBLK], f32, tag="fin", name="fin")
                        nc.vector.tensor_mul(out=fin, in0=accs[cc], in1=rec)
                        nc.vector.tensor_add(out=fin, in0=fin, in1=xq_c[nb_][:, cc, :])
                        nc.sync.dma_start(out=out_r[:, cc, nsl], in_=fin)

                def emit_slot(nb, mg):
                    st = stp.tile([P, GRP, NBLK], f32, tag="st", name="st")
                    for g in range(GRP):
                        nc.tensor.matmul(
                            st[:, g, :],
                            lhsT=k_c[mg][32 * g:32 * g + 32, g * P:(g + 1) * P],
                            rhs=q_c[nb][32 * g:32 * g + 32, :],
                            start=True,
                            stop=True,
                            tile_position=(32 * g, 0),
                        )
                    pt = ptp.tile([P, GRP, NBLK], bf16, tag="pt", name="pt")
                    nc.scalar.activation(out=pt, in_=st, func=Exp)
                    return pt

                prev = None

                def pump(nb, mg, pt):
                    nonlocal prev
                    if prev is not None:
                        pnb, pmg, ppt = prev
                        emit_av(pnb, pmg, ppt)
                        if pmg == NSLOT - 1:
                            emit_tail(pnb)
                    prev = (nb, mg, pt)

                def new_block(nb, pool):
                    a0 = pool.tile([P, NBLK], f32, tag="o0", name="a0")
                    a1 = pool.tile([P, NBLK], f32, tag="o1", name="a1")
                    dacc = dap.tile([P, GRP, NBLK], bf16, tag="da", name="dacc")
                    bstate[nb] = ([a0, a1], dacc, nb)

                # --- k/q projections (their psum banks free before AV) ---
                # pa(3) + wm(1) + S^T(4) = 8 banks; wm hosts warmup/filler
                # matmuls that keep the PE HAM window busy across DMA-wait
                # gaps (one >3.4us idle re-throttles the clock to 1.2GHz)
                with (
                    tc.tile_pool(name="pa", bufs=3, space="PSUM") as pap,
                    tc.tile_pool(name="wm", bufs=1, space="PSUM") as wmp,
                ):
                    def proj_chunk(dst, w4, bcol, src, name):
                        pp = pap.tile([P, CHUNK], f32, tag="pj", name=name)
                        for t in range(CT):
                            nc.tensor.matmul(
                                pp, lhsT=w4[:, t, :], rhs=src[:, t, :],
                                start=(t == 0), stop=(t == CT - 1),
                            )
                        # fused drain + per-partition bias add + f16 cast
                        nc.vector.tensor_scalar_add(dst, pp, bcol)

                    nc.gpsimd.dma_start(out=xq_c[0], in_=xq_r[:, :, 0:NBLK])
                    # ~3.5us of dummy matmuls while the first DMAs land:
                    # keeps the PE HAM busy-window warm so the projections
                    # run at 2.4GHz instead of the cold 1.2GHz
                    warm = const.tile([P, CHUNK], f16)
                    nc.vector.memset(warm, 0.0)
                    wp = wmp.tile([P, CHUNK], f32, tag="wm", name="warm")
                    for _ in range(18):
                        nc.tensor.matmul(wp, lhsT=warm[:, :P], rhs=warm,
                                         start=True, stop=True)
                    for ch in range(NCHUNKS):
                        proj_chunk(k_c[ch], wkt4, bk4, xh_c[ch], "kp")
                        if ch < QCHUNKS:
                            proj_chunk(q_c[ch], wqt4, bq4, xh_c[ch], "qp")
                        if ch < 6:
                            # filler: absorbs the next chunk's DMA wait
                            for _ in range(3):
                                nc.tensor.matmul(wp, lhsT=warm[:, :P], rhs=warm,
                                                 start=True, stop=True)

                # --- attention blocks, accumulators alternate (bufs=2) ---
                with tc.tile_pool(name="acc_ps", bufs=2, space="PSUM") as accp:
                    for nb in range(NBLKS):
                        new_block(nb, accp)
                        for mg in range(NSLOT):
                            pt = emit_slot(nb, mg)
                            pump(nb, mg, pt)
                    # final slot: the denominator partition-reduce cannot
                    # hide under a next block, so fold slots 0..6 early (a
                    # PE matmul into block 2's freed accumulator bank, runs
                    # during slot 7) and add slot 7's contribution after.
                    pnb, pmg, ppt = prev
                    accs, dacc, nb_ = bstate[pnb]
                    d2p = dnp.tile([P, 2, NBLK], bf16, tag="d2", name="d2p")
                    nc.vector.tensor_tensor(d2p, dacc[:, 0:2, :], dacc[:, 2:4, :], AluAdd)
                    d1p = dnp.tile([P, NBLK], bf16, tag="d1", name="d1p")
                    nc.vector.tensor_tensor(d1p, d2p[:, 0, :], d2p[:, 1, :], AluAdd)
                    den_ps = accp.tile([P, NBLK], f32, tag="o0", name="den_ps")
                    nc.tensor.matmul(den_ps[0:1, :], lhsT=ones_bf, rhs=d1p,
                                     start=True, stop=False)
                    emit_av(pnb, pmg, ppt, skip_dacc=True)
                    f1 = dnp.tile([P, 2, NBLK], bf16, tag="d2", name="f1")
                    nc.vector.tensor_tensor(f1, ppt[:, 0:2, :], ppt[:, 2:4, :], AluAdd)
                    f2 = dnp.tile([P, NBLK], bf16, tag="d1", name="f2")
                    nc.vector.tensor_tensor(f2, f1[:, 0, :], f1[:, 1, :], AluAdd)
                    nc.tensor.matmul(den_ps[0:1, :], lhsT=ones_bf, rhs=f2,
                                     start=False, stop=True)
                    rec1 = dnp.tile([1, NBLK], f32, tag="rc1", name="rec1")
                    nc.vector.reciprocal_approx_fast(rec1, den_ps[0:1, :])
                    recL = dnp.tile([P, NBLK], f32, tag="rec", name="recL")
                    nc.gpsimd.partition_broadcast(recL, rec1)
                    # inline fin path (emit_tail minus the denominator work)
                    nsl = slice(pnb * NBLK, (pnb + 1) * NBLK)
                    zsb = finp.tile([P, CT, NBLK], bf16, tag="zsb", name="zsb")
                    for cc in range(CT):
                        nc.vector.tensor_copy(out=zsb[:, cc, :], in_=accs[cc])
                    for co in range(CT):
                        for ci in range(CT):
                            nc.tensor.matmul(
                                accs[co],
                                lhsT=wvt[:, ci, co * P:(co + 1) * P],
                                rhs=zsb[:, ci, :],
                                start=(ci == 0),
                                stop=(ci == CT - 1),
                            )
                    for cc in range(CT):
                        fin = finp.tile([P, NBLK], f32, tag="fin", name="fin")
                        nc.vector.tensor_mul(out=fin, in0=accs[cc], in1=recL)
                        nc.vector.tensor_add(out=fin, in0=fin, in1=xq_c[nb_][:, cc, :])
                        nc.sync.dma_start(out=out_r[:, cc, nsl], in_=fin)
                    bstate.pop(pnb)

    nc.compile()
    return nc


def _get_prog():
    global _prog
    if _prog is None:
        _prog = _build()
    return _prog


def make_in_maps(inputs):
    x = np.ascontiguousarray(inputs["x"], dtype=np.float32).reshape(B, C, N)
    gamma = float(np.asarray(inputs["gamma"], np.float32).reshape(()))
    wq = np.asarray(inputs["wq"], np.float32)
    wk = np.asarray(inputs["wk"], np.float32)
    wv = np.asarray(inputs["wv"], np.float32)
    bq = np.asarray(inputs["bq"], np.float32)
    bk = np.asarray(inputs["bk"], np.float32)
    bv = np.asarray(inputs["bv"], np.float32)

    wqt4 = np.ascontiguousarray(np.tile(wq.T, (1, 4)).astype(np.float16))  # [C,128]
    wkt4 = np.ascontiguousarray(np.tile(wk.T, (1, 4)).astype(np.float16))
    import ml_dtypes

    wvt = np.ascontiguousarray((gamma * wv.T).astype(ml_dtypes.bfloat16))  # [C,C]
    bq4 = np.ascontiguousarray(np.tile(bq, 4).astype(np.float32))  # [128]
    bk4 = np.ascontiguousarray(np.tile(bk, 4).astype(np.float32))
    gbv = (gamma * bv).astype(np.float32)[:, None]  # [C,1]

    x_f16 = x.astype(np.float16)  # [B,C,N]
    in_maps = []
    for core in range(NCORES):
        b, h = divmod(core, 2)
        xq = x[b][:, h * NQ:(h + 1) * NQ] + gbv
        # own query half first (k and x^T use the same permutation)
        perm = np.r_[h * NQ:(h + 1) * NQ, (1 - h) * NQ:(2 - h) * NQ]
        xh_p = np.ascontiguousarray(x_f16[b][:, perm])
        xt_p = np.ascontiguousarray(x[b].T[perm].astype(ml_dtypes.bfloat16))
        in_maps.append(
            {
                "xh": xh_p,
                "xt": xt_p,
                "xq": np.ascontiguousarray(xq, dtype=np.float32),
                "wqt4": wqt4,
                "wkt4": wkt4,
                "wvt": wvt,
                "bq4": bq4,
                "bk4": bk4,
            }
        )
    return in_maps


def assemble(results):
    out = np.empty((B, C, N), np.float32)
    for core in range(NCORES):
        b, h = divmod(core, 2)
        out[b][:, h * NQ:(h + 1) * NQ] = results[core]["out"]
    return out.reshape(B, C, HH, WW)


def kernel(**inputs):
    from concourse.bass_utils import run_bass_kernel_spmd

    nc = _get_prog()
    in_maps = make_in_maps(inputs)
    res = run_bass_kernel_spmd(nc, in_maps, core_ids=list(range(NCORES)))
    return assemble(res.results)
